# revision 1
# baseline (speedup 1.0000x reference)
"""Causal self-attention kernel for 8 Trainium2 NeuronCores.

Problem (hardcoded): x [4, 2048, 768] f32, W [768, 2304] f32, b [2304] f32.
reference: qkv = x@W+b; 8 heads, head_dim 96; causal softmax attention.

Sharding: core c handles batch c//2 and heads 4*(c%2) .. 4*(c%2)+3
(data-parallel over batch x tensor-parallel over heads). Host shards
inputs / gathers outputs around one SPMD NEFF; no device collectives.

Per-core device program:
  - projection: qT,kT computed transposed [96, seq] (bias added as
    per-partition scalar), v computed natural [seq, 96] (bias added via a
    K=1 matmul against a ones row), with a ones column appended to v so
    the PV matmul also produces the softmax denominator.
  - attention in S^T layout: S^T[k,q] = kT.T @ qT per 512-wide q block;
    exp without max subtraction (logits ~N(0,1)); causal = skip upper
    blocks + mask-multiply on diagonal blocks; o^T accumulated in PSUM,
    PE-transposed back to natural layout and scaled by 1/denominator.

Matmul inputs use float32r (fp32 storage, 11-bit-mantissa matmul) for
full-rate PE throughput; host pre-rounds those inputs to f32r precision.
"""

import functools
from contextlib import ExitStack

import numpy as np

import concourse.bacc as bacc
import concourse.bass as bass
import concourse.mybir as mybir
import concourse.tile as tile
from concourse.bass_utils import run_bass_kernel_spmd
from concourse.masks import make_identity

F32 = mybir.dt.float32
F32R = mybir.dt.float32r

B, N, C, H = 4, 2048, 768, 8
D = C // H            # 96
NCORES = 8
LH = 4                # local heads per core
KC = C // 128         # 6 contraction chunks
NB = N // 512         # 4 seq blocks of 512
OUTC = LH * D         # 384
SCALE = float(1.0 / np.sqrt(np.float32(D)))
UNROLL2 = False
MASK_ENGINE = "gpsimd"   # gpsimd | vector
OSB_ENGINE = "vector"    # vector | scalar
QKADD_ENGINE = "vector"  # vector | scalar


@functools.lru_cache(maxsize=4)
def build(reps=1, use_f32r=True):
    MDT = F32R if use_f32r else F32
    nc = bacc.Bacc("TRN2", target_bir_lowering=False, debug=False,
                   num_devices=NCORES)
    xt_d = nc.dram_tensor("xt", [C, N], MDT, kind="ExternalInput")
    wqk_d = nc.dram_tensor("wqk", [C, 2 * LH * D], MDT, kind="ExternalInput")
    wv_d = nc.dram_tensor("wv", [C, LH * D], MDT, kind="ExternalInput")
    bqk_d = nc.dram_tensor("bqk", [D, 2 * LH], F32, kind="ExternalInput")
    bv_d = nc.dram_tensor("bv", [1, LH * D], MDT, kind="ExternalInput")
    out_d = nc.dram_tensor("out", [N, OUTC], F32, kind="ExternalOutput")

    xt_v = xt_d.ap().rearrange("(kc p) n -> p kc n", p=128)
    wqk_v = wqk_d.ap().rearrange("(kc p) m -> p kc m", p=128)
    wv_v = wv_d.ap().rearrange("(kc p) m -> p kc m", p=128)
    out_v = out_d.ap().rearrange("(qq t p) c -> qq p t c", t=4, p=128)

    with tile.TileContext(nc) as tc, ExitStack() as ctx:
        const = ctx.enter_context(tc.tile_pool(name="const", bufs=1))
        wpool = ctx.enter_context(tc.tile_pool(name="w", bufs=1))
        xpool = ctx.enter_context(tc.tile_pool(name="x", bufs=3))
        qkpool = ctx.enter_context(tc.tile_pool(name="qk", bufs=NB))
        vpool = ctx.enter_context(tc.tile_pool(name="vaug", bufs=NB))
        ppool = ctx.enter_context(tc.tile_pool(name="p", bufs=6))
        opool = ctx.enter_context(tc.tile_pool(name="osb", bufs=3))
        rpool = ctx.enter_context(tc.tile_pool(name="r", bufs=4))
        spool = ctx.enter_context(tc.tile_pool(name="stage", bufs=3))
        ps_proj = ctx.enter_context(
            tc.tile_pool(name="ps_proj", bufs=2, space="PSUM"))
        ps_s = ctx.enter_context(
            tc.tile_pool(name="ps_s", bufs=2, space="PSUM"))
        ps_o = ctx.enter_context(
            tc.tile_pool(name="ps_o", bufs=2, space="PSUM"))
        ps_t = ps_proj

        # one-time constants
        identity = const.tile([128, 128], F32)
        make_identity(nc, identity[:])
        # mask[p, f] = 1.0 if f >= p else 0.0   (keep q >= k on diag blocks)
        mask = const.tile([128, 128], F32)
        nc.gpsimd.memset(mask[:], 1.0)
        nc.gpsimd.affine_select(
            out=mask[:], in_=mask[:], compare_op=mybir.AluOpType.is_ge,
            fill=0.0, base=0, pattern=[[1, 128]], channel_multiplier=-1)
        ones = const.tile([1, 128], F32)
        nc.gpsimd.memset(ones[:], 1.0)
        ones_r = const.tile([1, 128], MDT)
        nc.vector.tensor_copy(ones_r[:], ones[:])
        vones = const.tile([128, 4, LH, 1], F32)
        nc.gpsimd.memset(vones[:], 1.0)

        wqk_sb = wpool.tile([128, KC, 2 * LH * D], MDT, tag="wqk")
        wv_sb = wpool.tile([128, KC, LH * D], MDT, tag="wv")
        for kc in range(KC):
            nc.sync.dma_start(wqk_sb[:, kc, :], wqk_v[:, kc, :])
            nc.sync.dma_start(wv_sb[:, kc, :], wv_v[:, kc, :])
        bqk_sb = wpool.tile([D, 2 * LH], F32, tag="bqk")
        nc.sync.dma_start(bqk_sb[:], bqk_d.ap())
        bv_sb = wpool.tile([1, LH * D], MDT, tag="bv")
        nc.sync.dma_start(bv_sb[:], bv_d.ap())

        def body():
            qk_tiles = []
            va_tiles = []
            for nb in range(NB):
                # ---- load x block (transposed layout) ----
                xt_sb = xpool.tile([128, KC, 512], MDT, tag="xt")
                for kc in range(KC):
                    nc.sync.dma_start(
                        xt_sb[:, kc, :],
                        xt_v[:, kc, nb * 512:(nb + 1) * 512])

                # ---- v projection (natural layout) for this seq block ----
                va = vpool.tile([128, 4, LH, D + 1], MDT, tag="va")
                nc.vector.tensor_copy(va[:, :, :, D:D + 1], vones[:])
                for mt in range(4):
                    vps = ps_proj.tile([128, 512], F32, tag="proj")
                    # bias first (covers all columns, start=True)
                    nc.tensor.matmul(
                        vps[:, 0:LH * D], ones_r[:, :], bv_sb[:, :],
                        start=True, stop=False)
                    for kc in range(KC):
                        nc.tensor.matmul(
                            vps[:, 0:LH * D],
                            xt_sb[:, kc, mt * 128:(mt + 1) * 128],
                            wv_sb[:, kc, :],
                            start=False, stop=(kc == KC - 1))
                    nc.vector.tensor_copy(
                        va[:, mt, :, 0:D],
                        vps[:, 0:LH * D].rearrange("p (h d) -> p h d", h=LH))
                va_tiles.append(va)

                # ---- q,k projection (transposed layout) for this block ----
                qk_h = []
                for hh in range(LH):
                    qk = qkpool.tile([D, 2, 512], MDT, tag=f"qk{hh}")
                    for t in range(2):
                        m = 2 * hh + t
                        qps = ps_proj.tile([128, 512], F32, tag="proj")
                        for kc in range(KC):
                            nc.tensor.matmul(
                                qps[0:D, :],
                                wqk_sb[:, kc, m * D:(m + 1) * D],
                                xt_sb[:, kc, :],
                                start=(kc == 0), stop=(kc == KC - 1))
                        nc.vector.tensor_scalar_add(
                            qk[:, t, :], qps[0:D, :], bqk_sb[:, m:m + 1])
                    qk_h.append(qk)
                qk_tiles.append(qk_h)

                # ---- attention for q block Q = nb, all local heads ----
                Q = nb
                for h in range(LH):
                    ops = ps_o.tile([D + 1, 512], F32, tag="o")
                    jmax = 4 * Q + 3
                    for pr in range((jmax + 1) // 2):
                        sps = ps_s.tile([128, 1024], F32, tag="s")
                        info = []
                        for idx in range(2):
                            j = 2 * pr + idx
                            qoff = max(512 * Q, 128 * j)
                            width = 512 * (Q + 1) - qoff
                            info.append((j, qoff, width))
                            nc.tensor.matmul(
                                sps[:, idx * 512:idx * 512 + width],
                                qk_tiles[j // 4][h][
                                    :, 1,
                                    (j % 4) * 128:(j % 4) * 128 + 128],
                                qk_tiles[Q][h][
                                    :, 0,
                                    qoff - 512 * Q:qoff - 512 * Q + width],
                                start=True, stop=True)
                        pt = ppool.tile([128, 1024], MDT, tag="p")
                        nc.scalar.activation(
                            pt[:, 0:512 + info[1][2]], sps[:, 0:512 + info[1][2]],
                            mybir.ActivationFunctionType.Exp, scale=SCALE)
                        for idx, (j, qoff, width) in enumerate(info):
                            if j >= 4 * Q:  # diagonal block: causal mask
                                if MASK_ENGINE == "gpsimd":
                                    nc.gpsimd.affine_select(
                                        out=pt[:, idx * 512:idx * 512 + 128],
                                        in_=pt[:, idx * 512:idx * 512 + 128],
                                        compare_op=mybir.AluOpType.is_ge,
                                        fill=0.0, base=0, pattern=[[1, 128]],
                                        channel_multiplier=-1)
                                else:
                                    nc.vector.tensor_mul(
                                        pt[:, idx * 512:idx * 512 + 128],
                                        pt[:, idx * 512:idx * 512 + 128],
                                        mask[:])
                            nc.tensor.matmul(
                                ops[:, qoff - 512 * Q:512],
                                va_tiles[j // 4][:, j % 4, h, :],
                                pt[:, idx * 512:idx * 512 + width],
                                start=(j == 0), stop=(j == jmax))
                    o_sb = opool.tile([D + 1, 512], F32, tag="osb")
                    if OSB_ENGINE == "vector":
                        nc.vector.tensor_copy(o_sb[:], ops[:])
                    else:
                        nc.scalar.copy(o_sb[:], ops[:])
                    stage = spool.tile([128, 4, D], F32, tag="stage")
                    for t in range(4):
                        tps = ps_t.tile([128, 128], F32, tag="proj")
                        nc.tensor.transpose(
                            tps[:, 0:D + 1], o_sb[:, t * 128:(t + 1) * 128],
                            identity[0:D + 1, 0:D + 1])
                        rr = rpool.tile([128, 1], F32, tag="r")
                        nc.vector.reciprocal(rr[:], tps[:, D:D + 1])
                        nc.vector.tensor_scalar_mul(
                            stage[:, t, :], tps[:, 0:D], rr[:])
                    nc.sync.dma_start(
                        out_v[Q, :, :, h * D:(h + 1) * D], stage[:])

        if reps == 1:
            body()
        else:
            with tc.For_i(0, reps, 1):
                body()
                if UNROLL2:
                    body()

    nc.compile()
    return nc


def f32r_round(a):
    """Round fp32 array to f32r precision (11-bit mantissa, RNE)."""
    u = np.ascontiguousarray(a, dtype=np.float32).view(np.uint32)
    u = (u + 0x7FF + ((u >> 12) & 1)) & np.uint32(0xFFFFF000)
    return u.view(np.float32)


def shard_inputs(x, W, b, use_f32r=True):
    """Full inputs -> per-core in_maps (numpy, fp32)."""
    x = np.asarray(x, dtype=np.float32)
    W = np.asarray(W, dtype=np.float32)
    b = np.asarray(b, dtype=np.float32)
    if use_f32r:
        # round once globally (elementwise, commutes with slicing below)
        x = f32r_round(x)
        W = f32r_round(W)
    rnd = lambda a: np.ascontiguousarray(a, dtype=np.float32)
    in_maps = []
    for c in range(NCORES):
        bc, g = divmod(c, 2)
        h0 = g * LH
        qcols = [W[:, 0 * C + (h0 + h) * D:0 * C + (h0 + h + 1) * D]
                 for h in range(LH)]
        kcols = [W[:, 1 * C + (h0 + h) * D:1 * C + (h0 + h + 1) * D]
                 for h in range(LH)]
        vcols = [W[:, 2 * C + (h0 + h) * D:2 * C + (h0 + h + 1) * D]
                 for h in range(LH)]
        wqk = np.concatenate(
            [m for h in range(LH) for m in (qcols[h], kcols[h])], axis=1)
        wv = np.concatenate(vcols, axis=1)
        bqk = np.stack(
            [b[t * C + (h0 + h) * D:t * C + (h0 + h + 1) * D]
             for h in range(LH) for t in (0, 1)], axis=1)
        bv = np.concatenate(
            [b[2 * C + (h0 + h) * D:2 * C + (h0 + h + 1) * D]
             for h in range(LH)])[None, :]
        in_maps.append({
            "xt": rnd(x[bc].T),
            "wqk": rnd(wqk),
            "wv": rnd(wv),
            "bqk": np.ascontiguousarray(bqk),
            "bv": rnd(bv),
        })
    return in_maps


def gather_outputs(results):
    """Per-core results -> full [B, N, C] output."""
    out = np.empty((B, N, C), dtype=np.float32)
    for c in range(NCORES):
        bc, g = divmod(c, 2)
        out[bc, :, g * OUTC:(g + 1) * OUTC] = results[c]["out"]
    return out


def kernel(x, W, b):
    nc = build(reps=1, use_f32r=True)
    in_maps = shard_inputs(x, W, b, use_f32r=True)
    res = run_bass_kernel_spmd(nc, in_maps, core_ids=list(range(NCORES)))
    return gather_outputs(res.results)



# revision 5
# speedup vs baseline: 1.5304x; 1.5304x over previous
"""Causal self-attention kernel for 8 Trainium2 NeuronCores.

Problem (hardcoded): x [4, 2048, 768] f32, W [768, 2304] f32, b [2304] f32.
reference: qkv = x@W+b; 8 heads, head_dim 96; causal softmax attention.

Sharding: core c handles batch c//2 and heads 4*(c%2) .. 4*(c%2)+3
(data-parallel over batch x tensor-parallel over heads). Host shards
inputs / gathers outputs around one SPMD NEFF; no device collectives.

Per-core device program (PE-pipelined schedule):
  - projection: qT,kT computed transposed [96, seq] in f32r (bias added on
    eviction), v computed natural [seq, 96] with a ones column appended so
    the PV matmul also produces the softmax denominator. Attention operands
    (q,k,v,P,o) are stored bf16 (full-rate PE for any width, cheap
    transposes); accumulation stays f32 in PSUM.
  - attention in S^T layout per 512-wide q block: S^T[k,q] = kT.T @ qT,
    exp without max subtraction (logits ~N(0,1)), causal = skip upper
    blocks + affine_select on diagonal blocks; o^T accumulated in PSUM,
    PE-transposed back and scaled by 1/denominator.
  - scheduling: the Tensor engine's clock ramps to 2.4 GHz only under
    sustained execution, so the emission order keeps PE gapless: S and PV
    matmuls are software-pipelined with a 2-pair lag (PV(k) is emitted
    after S(k+2), hiding the exp->mask latency), the next block's
    projection matmuls are woven one-at-a-time into the attention pair
    loop as PE filler (covering the ACT engine's exp deficit), and each
    head's o-postprocessing is deferred into the next head's pair loop.
    Input DMAs issue from the sync queue, output DMAs from the gpsimd
    queue, so input prefetch is never stuck behind output drains.

Matmul inputs for the projection use float32r (fp32 storage, 11-bit
mantissa); host pre-rounds those inputs to f32r precision.
"""

import functools
from collections import deque
from contextlib import ExitStack

import numpy as np

import concourse.bacc as bacc
import concourse.bass as bass
import concourse.mybir as mybir
import concourse.tile as tile
from concourse.bass_utils import run_bass_kernel_spmd
from concourse.masks import make_identity

F32 = mybir.dt.float32
F32R = mybir.dt.float32r
BF16 = mybir.dt.bfloat16

B, N, C, H = 4, 2048, 768, 8
D = C // H            # 96
NCORES = 8
LH = 4                # local heads per core
KC = C // 128         # 6 contraction chunks
NB = N // 512         # 4 seq blocks of 512
OUTC = LH * D         # 384
SCALE = float(1.0 / np.sqrt(np.float32(D)))
UNROLL2 = False


@functools.lru_cache(maxsize=4)
def build(reps=1, use_f32r=True):
    MDT = F32R if use_f32r else F32
    nc = bacc.Bacc("TRN2", target_bir_lowering=False, debug=False,
                   num_devices=NCORES)
    xt_d = nc.dram_tensor("xt", [C, N], MDT, kind="ExternalInput")
    wqk_d = nc.dram_tensor("wqk", [C, 2 * LH * D], MDT, kind="ExternalInput")
    wv_d = nc.dram_tensor("wv", [C, LH * D], MDT, kind="ExternalInput")
    bqk_d = nc.dram_tensor("bqk", [D, 2 * LH], F32, kind="ExternalInput")
    bv_d = nc.dram_tensor("bv", [1, LH * D], MDT, kind="ExternalInput")
    out_d = nc.dram_tensor("out", [N, OUTC], F32, kind="ExternalOutput")

    xt_v = xt_d.ap().rearrange("(kc p) n -> p kc n", p=128)
    wqk_v = wqk_d.ap().rearrange("(kc p) m -> p kc m", p=128)
    wv_v = wv_d.ap().rearrange("(kc p) m -> p kc m", p=128)
    out_v = out_d.ap().rearrange("(qq t p) c -> qq p t c", t=4, p=128)

    with tile.TileContext(nc) as tc, ExitStack() as ctx:
        const = ctx.enter_context(tc.tile_pool(name="const", bufs=1))
        wpool = ctx.enter_context(tc.tile_pool(name="w", bufs=1))
        xpool = ctx.enter_context(tc.tile_pool(name="x", bufs=3))
        qkpool = ctx.enter_context(tc.tile_pool(name="qk", bufs=NB + 1))
        vpool = ctx.enter_context(tc.tile_pool(name="vaug", bufs=NB + 1))
        ppool = ctx.enter_context(tc.tile_pool(name="p", bufs=6))
        opool = ctx.enter_context(tc.tile_pool(name="osb", bufs=3))
        rpool = ctx.enter_context(tc.tile_pool(name="r", bufs=4))
        spool = ctx.enter_context(tc.tile_pool(name="stage", bufs=3))
        ps_proj = ctx.enter_context(
            tc.tile_pool(name="ps_proj", bufs=2, space="PSUM"))
        ps_s = ctx.enter_context(
            tc.tile_pool(name="ps_s", bufs=2, space="PSUM"))
        ps_o = ctx.enter_context(
            tc.tile_pool(name="ps_o", bufs=2, space="PSUM"))

        # one-time constants
        identity_bf = const.tile([128, 128], BF16)
        make_identity(nc, identity_bf[:])
        ones = const.tile([1, 128], F32)
        nc.gpsimd.memset(ones[:], 1.0)
        ones_r = const.tile([1, 128], MDT)
        nc.vector.tensor_copy(ones_r[:], ones[:])

        wqk_sb = wpool.tile([128, KC, 2 * LH * D], MDT, tag="wqk")
        wv_sb = wpool.tile([128, KC, LH * D], MDT, tag="wv")
        for kc in range(KC):
            nc.sync.dma_start(wqk_sb[:, kc, :], wqk_v[:, kc, :])
            nc.sync.dma_start(wv_sb[:, kc, :], wv_v[:, kc, :])
        bqk_sb = wpool.tile([D, 2 * LH], F32, tag="bqk")
        nc.sync.dma_start(bqk_sb[:], bqk_d.ap())
        bv_sb = wpool.tile([1, LH * D], MDT, tag="bv")
        nc.sync.dma_start(bv_sb[:], bv_d.ap())

        def body():
            filler = deque()

            def weave(n=1):
                for _ in range(n):
                    if filler:
                        filler.popleft()()

            def drain():
                while filler:
                    filler.popleft()()

            xt_sb = [None] * NB
            va_tiles = [None] * NB
            qk_tiles = [[None] * LH for _ in range(NB)]
            pend_o = [None]

            def load_x(nb):
                t = xpool.tile([128, KC, 512], MDT, tag="xt", name="xt_sb")
                for kc in range(KC):
                    nc.sync.dma_start(
                        t[:, kc, :], xt_v[:, kc, nb * 512:(nb + 1) * 512])
                xt_sb[nb] = t

            def vproj_units(nb):
                """PE-filler closures for the v projection of seq block nb."""
                va = vpool.tile([128, 4, LH, D + 1], BF16, tag="va", name="va")
                va_tiles[nb] = va
                units = []
                box = {}

                def first(mt):
                    def f():
                        if mt == 0:
                            nc.gpsimd.memset(va[:, :, :, D:D + 1], 1.0)
                        vps = ps_proj.tile([128, 512], F32, tag="proj",
                                           name="vps")
                        box[mt] = vps
                        nc.tensor.matmul(
                            vps[:, 0:OUTC], ones_r[:, :], bv_sb[:, :],
                            start=True, stop=False)
                        nc.tensor.matmul(
                            vps[:, 0:OUTC],
                            xt_sb[nb][:, 0, mt * 128:(mt + 1) * 128],
                            wv_sb[:, 0, :], start=False, stop=False)
                    return f

                def mid(mt, kc):
                    def f():
                        nc.tensor.matmul(
                            box[mt][:, 0:OUTC],
                            xt_sb[nb][:, kc, mt * 128:(mt + 1) * 128],
                            wv_sb[:, kc, :],
                            start=False, stop=(kc == KC - 1))
                        if kc == KC - 1:
                            nc.vector.tensor_copy(
                                va[:, mt, :, 0:D],
                                box[mt][:, 0:OUTC].rearrange(
                                    "p (h d) -> p h d", h=LH))
                    return f

                for mt in range(4):
                    units.append(first(mt))
                    for kc in range(1, KC):
                        units.append(mid(mt, kc))
                return units

            def qkproj_units(nb, hh):
                """PE-filler closures for the q,k projection of (block, head)."""
                qk = qkpool.tile([D, 2, 512], BF16, tag=f"qk{hh}",
                                 name=f"qk{hh}")
                qk_tiles[nb][hh] = qk
                units = []
                box = {}

                def first(t):
                    m = 2 * hh + t

                    def f():
                        qps = ps_proj.tile([128, 512], F32, tag="proj",
                                           name="qps")
                        box[t] = qps
                        nc.tensor.matmul(
                            qps[0:D, :], wqk_sb[:, 0, m * D:(m + 1) * D],
                            xt_sb[nb][:, 0, :], start=True, stop=False)
                    return f

                def mid(t, kc):
                    m = 2 * hh + t

                    def f():
                        nc.tensor.matmul(
                            box[t][0:D, :], wqk_sb[:, kc, m * D:(m + 1) * D],
                            xt_sb[nb][:, kc, :],
                            start=False, stop=(kc == KC - 1))
                        if kc == KC - 1:
                            nc.vector.tensor_scalar_add(
                                qk[:, t, :], box[t][0:D, :],
                                bqk_sb[:, m:m + 1])
                    return f

                for t in range(2):
                    units.append(first(t))
                    for kc in range(1, KC):
                        units.append(mid(t, kc))
                return units

            def attn_head(Q, h):
                jmax = 4 * Q + 3
                P = 2 * Q + 2
                ops = ps_o.tile([D + 1, 512], F32, tag="o", name="ops")
                pt_l, info_l = {}, {}

                def S_pair(pr):
                    sps = ps_s.tile([128, 1024], F32, tag="s", name="sps")
                    info = []
                    for idx in range(2):
                        j = 2 * pr + idx
                        qoff = max(512 * Q, 128 * j)
                        width = 512 * (Q + 1) - qoff
                        info.append((j, qoff, width))
                        nc.tensor.matmul(
                            sps[:, idx * 512:idx * 512 + width],
                            qk_tiles[j // 4][h][
                                :, 1, (j % 4) * 128:(j % 4) * 128 + 128],
                            qk_tiles[Q][h][
                                :, 0, qoff - 512 * Q:qoff - 512 * Q + width],
                            start=True, stop=True)
                    info_l[pr] = info
                    pt = ppool.tile([128, 1024], BF16, tag="p", name="pt")
                    pt_l[pr] = pt
                    nc.scalar.activation(
                        pt[:, 0:512 + info[1][2]], sps[:, 0:512 + info[1][2]],
                        mybir.ActivationFunctionType.Exp, scale=SCALE)
                    for idx, (j, qoff, width) in enumerate(info):
                        if j >= 4 * Q:  # diagonal block: causal mask
                            nc.gpsimd.affine_select(
                                out=pt[:, idx * 512:idx * 512 + 128],
                                in_=pt[:, idx * 512:idx * 512 + 128],
                                compare_op=mybir.AluOpType.is_ge,
                                fill=0.0, base=0, pattern=[[1, 128]],
                                channel_multiplier=-1)

                def PV_pair(pr):
                    for idx, (j, qoff, width) in enumerate(info_l[pr]):
                        nc.tensor.matmul(
                            ops[:, qoff - 512 * Q:512],
                            va_tiles[j // 4][:, j % 4, h, :],
                            pt_l[pr][:, idx * 512:idx * 512 + width],
                            start=(j == 0), stop=(j == jmax))

                for pr in range(P):
                    S_pair(pr)
                    if pr == 1 and pend_o[0] is not None:
                        pend_o[0]()
                        pend_o[0] = None
                    if pr >= 2:
                        PV_pair(pr - 2)
                    weave()
                n_tail = 2 if P <= 3 else 1
                weave(n_tail)
                PV_pair(P - 2)
                weave(n_tail)
                PV_pair(P - 1)

                def process_o():
                    o_sb = opool.tile([D + 1, 512], BF16, tag="osb",
                                      name="o_sb")
                    nc.vector.tensor_copy(o_sb[:], ops[:])
                    # row stride D+2 keeps each bf16 PSUM slice 4B-aligned
                    tps = ps_proj.tile([128, 4, D + 2], BF16, tag="proj",
                                       name="tps")
                    for t in range(4):
                        nc.tensor.transpose(
                            tps[:, t, 0:D + 1], o_sb[:, t * 128:(t + 1) * 128],
                            identity_bf[0:D + 1, 0:D + 1])
                    rr = rpool.tile([128, 4, 1], F32, tag="r", name="rr")
                    nc.vector.reciprocal(rr[:], tps[:, :, D:D + 1])
                    stage = spool.tile([128, 4, D], F32, tag="stage",
                                       name="stage")
                    for t in range(4):
                        nc.vector.tensor_scalar_mul(
                            stage[:, t, :], tps[:, t, 0:D], rr[:, t, :])
                    nc.gpsimd.dma_start(
                        out_v[Q, :, :, h * D:(h + 1) * D], stage[:])

                pend_o[0] = process_o

            # ---- body schedule ----
            load_x(0)
            load_x(1)
            for u in vproj_units(0):
                u()
            for hh in range(LH):
                for u in qkproj_units(0, hh):
                    u()
            for Q in range(NB):
                if Q + 2 < NB:
                    load_x(Q + 2)
                for h in range(LH):
                    if Q + 1 < NB:
                        if h == 0:
                            filler.extend(vproj_units(Q + 1))
                            filler.extend(qkproj_units(Q + 1, 0))
                        elif Q + 1 < NB - 1:
                            filler.extend(qkproj_units(Q + 1, h))
                        # when Q+1 is the last block, its qk head 1..3
                        # projections are deferred into that block itself
                    else:
                        if h + 1 < LH:
                            filler.extend(qkproj_units(Q, h + 1))
                    attn_head(Q, h)
                    if Q == NB - 1:
                        drain()
                drain()
            if pend_o[0] is not None:
                pend_o[0]()
                pend_o[0] = None

        if reps == 1:
            body()
        else:
            with tc.For_i(0, reps, 1):
                body()
                if UNROLL2:
                    body()

    nc.compile()
    return nc


def f32r_round(a):
    """Round fp32 array to f32r precision (11-bit mantissa, RNE)."""
    u = np.ascontiguousarray(a, dtype=np.float32).view(np.uint32)
    u = (u + 0x7FF + ((u >> 12) & 1)) & np.uint32(0xFFFFF000)
    return u.view(np.float32)


def shard_inputs(x, W, b, use_f32r=True):
    """Full inputs -> per-core in_maps (numpy, fp32)."""
    x = np.asarray(x, dtype=np.float32)
    W = np.asarray(W, dtype=np.float32)
    b = np.asarray(b, dtype=np.float32)
    if use_f32r:
        # round once globally (elementwise, commutes with slicing below)
        x = f32r_round(x)
        W = f32r_round(W)
    rnd = lambda a: np.ascontiguousarray(a, dtype=np.float32)
    in_maps = []
    for c in range(NCORES):
        bc, g = divmod(c, 2)
        h0 = g * LH
        qcols = [W[:, 0 * C + (h0 + h) * D:0 * C + (h0 + h + 1) * D]
                 for h in range(LH)]
        kcols = [W[:, 1 * C + (h0 + h) * D:1 * C + (h0 + h + 1) * D]
                 for h in range(LH)]
        vcols = [W[:, 2 * C + (h0 + h) * D:2 * C + (h0 + h + 1) * D]
                 for h in range(LH)]
        wqk = np.concatenate(
            [m for h in range(LH) for m in (qcols[h], kcols[h])], axis=1)
        wv = np.concatenate(vcols, axis=1)
        bqk = np.stack(
            [b[t * C + (h0 + h) * D:t * C + (h0 + h + 1) * D]
             for h in range(LH) for t in (0, 1)], axis=1)
        bv = np.concatenate(
            [b[2 * C + (h0 + h) * D:2 * C + (h0 + h + 1) * D]
             for h in range(LH)])[None, :]
        in_maps.append({
            "xt": rnd(x[bc].T),
            "wqk": rnd(wqk),
            "wv": rnd(wv),
            "bqk": np.ascontiguousarray(bqk),
            "bv": rnd(bv),
        })
    return in_maps


def gather_outputs(results):
    """Per-core results -> full [B, N, C] output."""
    out = np.empty((B, N, C), dtype=np.float32)
    for c in range(NCORES):
        bc, g = divmod(c, 2)
        out[bc, :, g * OUTC:(g + 1) * OUTC] = results[c]["out"]
    return out


def kernel(x, W, b):
    nc = build(reps=1, use_f32r=True)
    in_maps = shard_inputs(x, W, b, use_f32r=True)
    res = run_bass_kernel_spmd(nc, in_maps, core_ids=list(range(NCORES)))
    return gather_outputs(res.results)


# revision 12
# speedup vs baseline: 1.6468x; 1.0760x over previous
"""Causal self-attention kernel for 8 Trainium2 NeuronCores.

Problem (hardcoded): x [4, 2048, 768] f32, W [768, 2304] f32, b [2304] f32.
reference: qkv = x@W+b; 8 heads, head_dim 96; causal softmax attention.

Sharding: core c handles batch c//2 and heads 4*(c%2) .. 4*(c%2)+3
(data-parallel over batch x tensor-parallel over heads). Host shards
inputs / gathers outputs around one SPMD NEFF; no device collectives.

Per-core device program (PE-pipelined schedule):
  - projection: qT,kT computed transposed [96, seq] in f32r (bias added on
    eviction), v computed natural [seq, 96] with a ones column appended so
    the PV matmul also produces the softmax denominator. Attention operands
    (q,k,v,P,o) are stored bf16 (full-rate PE for any width, cheap
    transposes); accumulation stays f32 in PSUM.
  - attention in S^T layout per 512-wide q block: S^T[k,q] = kT.T @ qT,
    exp without max subtraction (logits ~N(0,1)), causal = skip upper
    blocks + affine_select on diagonal blocks; o^T accumulated in PSUM,
    PE-transposed back and scaled by 1/denominator.
  - scheduling: the Tensor engine's clock ramps to 2.4 GHz only under
    sustained execution, so the emission order keeps PE gapless: S and PV
    matmuls are software-pipelined with a 2-pair lag (PV(k) is emitted
    after S(k+2), hiding the exp->mask latency), the next block's
    projection matmuls are woven one-at-a-time into the attention pair
    loop as PE filler (covering the ACT engine's exp deficit), and each
    head's o-postprocessing is deferred into the next head's pair loop.
    Input DMAs issue from the sync queue, output DMAs from the gpsimd
    queue, so input prefetch is never stuck behind output drains.

Matmul inputs for the projection use float32r (fp32 storage, 11-bit
mantissa); host pre-rounds those inputs to f32r precision.
"""

import functools
from collections import deque
from contextlib import ExitStack

import numpy as np

import concourse.bacc as bacc
import concourse.bass as bass
import concourse.mybir as mybir
import concourse.tile as tile
from concourse.bass_utils import run_bass_kernel_spmd
from concourse.masks import make_identity

F32 = mybir.dt.float32
F32R = mybir.dt.float32r
BF16 = mybir.dt.bfloat16

B, N, C, H = 4, 2048, 768, 8
D = C // H            # 96
NCORES = 8
LH = 4                # local heads per core
KC = C // 128         # 6 contraction chunks
NB = N // 512         # 4 seq blocks of 512
OUTC = LH * D         # 384
SCALE = float(1.0 / np.sqrt(np.float32(D)))
UNROLL2 = False


@functools.lru_cache(maxsize=8)
def build(reps=1, use_f32r=True, unroll=False):
    MDT = F32R if use_f32r else F32
    nc = bacc.Bacc("TRN2", target_bir_lowering=False, debug=False,
                   num_devices=NCORES)
    xt_d = nc.dram_tensor("xt", [C, N], MDT, kind="ExternalInput")
    wqk_d = nc.dram_tensor("wqk", [C, 2 * LH * D], MDT, kind="ExternalInput")
    wv_d = nc.dram_tensor("wv", [C, LH * D], MDT, kind="ExternalInput")
    bqk_d = nc.dram_tensor("bqk", [D, 2 * LH], F32, kind="ExternalInput")
    bv_d = nc.dram_tensor("bv", [1, LH * D], MDT, kind="ExternalInput")
    out_d = nc.dram_tensor("out", [N, OUTC], F32, kind="ExternalOutput")

    xt_v = xt_d.ap().rearrange("(kc p) n -> p kc n", p=128)
    wqk_v = wqk_d.ap().rearrange("(kc p) m -> p kc m", p=128)
    wv_v = wv_d.ap().rearrange("(kc p) m -> p kc m", p=128)
    out_v = out_d.ap().rearrange("(qq t p) c -> qq p t c", t=4, p=128)

    with tile.TileContext(nc) as tc, ExitStack() as ctx:
        const = ctx.enter_context(tc.tile_pool(name="const", bufs=1))
        wpool = ctx.enter_context(tc.tile_pool(name="w", bufs=1))
        xpool = ctx.enter_context(tc.tile_pool(name="x", bufs=3))
        qkpool = ctx.enter_context(tc.tile_pool(name="qk", bufs=NB + 1))
        vpool = ctx.enter_context(tc.tile_pool(name="vaug", bufs=NB + 1))
        ppool = ctx.enter_context(tc.tile_pool(name="p", bufs=6))
        opool = ctx.enter_context(tc.tile_pool(name="osb", bufs=3))
        rpool = ctx.enter_context(tc.tile_pool(name="r", bufs=4))
        spool = ctx.enter_context(tc.tile_pool(name="stage", bufs=3))
        ps_proj = ctx.enter_context(
            tc.tile_pool(name="ps_proj", bufs=2, space="PSUM"))
        ps_s = ctx.enter_context(
            tc.tile_pool(name="ps_s", bufs=2, space="PSUM"))
        ps_o = ctx.enter_context(
            tc.tile_pool(name="ps_o", bufs=2, space="PSUM"))

        # one-time constants
        identity_bf = const.tile([128, 128], BF16)
        make_identity(nc, identity_bf[:])
        ones = const.tile([1, 128], F32)
        nc.gpsimd.memset(ones[:], 1.0)
        ones_r = const.tile([1, 128], MDT)
        nc.vector.tensor_copy(ones_r[:], ones[:])

        wqk_sb = wpool.tile([128, KC, 2 * LH * D], MDT, tag="wqk")
        wv_sb = wpool.tile([128, KC, LH * D], MDT, tag="wv")
        for kc in range(KC):
            nc.sync.dma_start(wqk_sb[:, kc, :], wqk_v[:, kc, :])
            nc.sync.dma_start(wv_sb[:, kc, :], wv_v[:, kc, :])
        bqk_sb = wpool.tile([D, 2 * LH], F32, tag="bqk")
        nc.sync.dma_start(bqk_sb[:], bqk_d.ap())
        bv_sb = wpool.tile([1, LH * D], MDT, tag="bv")
        nc.sync.dma_start(bv_sb[:], bv_d.ap())
        # bv broadcast to all 128 partitions (once) so the v bias can be
        # applied during PSUM eviction instead of a per-block matmul
        bv_ps = ps_proj.tile([128, 512], F32, tag="proj", name="bv_ps")
        nc.tensor.matmul(bv_ps[:, 0:OUTC], ones_r[:, :], bv_sb[:, :],
                         start=True, stop=True)
        bv_rep = wpool.tile([128, LH, D], F32, tag="bvrep")
        nc.vector.tensor_copy(
            bv_rep[:], bv_ps[:, 0:OUTC].rearrange("p (h d) -> p h d", h=LH))

        def body():
            filler = deque()

            def weave(n=1):
                for _ in range(n):
                    if filler:
                        filler.popleft()()

            def drain():
                while filler:
                    filler.popleft()()

            xt_sb = [None] * NB
            va_tiles = [None] * NB
            qk_tiles = [[None] * LH for _ in range(NB)]
            pend_o = [None]

            def load_x(nb):
                t = xpool.tile([128, KC, 512], MDT, tag="xt", name="xt_sb")
                for kc in range(KC):
                    nc.sync.dma_start(
                        t[:, kc, :], xt_v[:, kc, nb * 512:(nb + 1) * 512])
                xt_sb[nb] = t

            def vproj_units(nb):
                """PE-filler closures for the v projection of seq block nb."""
                va = vpool.tile([128, 4, LH, D + 1], BF16, tag="va", name="va")
                va_tiles[nb] = va
                units = []
                box = {}

                def first(mt):
                    def f():
                        if mt == 0:
                            nc.gpsimd.memset(va[:, :, :, D:D + 1], 1.0)
                        vps = ps_proj.tile([128, 512], F32, tag="proj",
                                           name="vps")
                        box[mt] = vps
                        nc.tensor.matmul(
                            vps[:, 0:OUTC],
                            xt_sb[nb][:, 0, mt * 128:(mt + 1) * 128],
                            wv_sb[:, 0, :], start=True, stop=False)
                    return f

                def mid(mt, kc):
                    def f():
                        nc.tensor.matmul(
                            box[mt][:, 0:OUTC],
                            xt_sb[nb][:, kc, mt * 128:(mt + 1) * 128],
                            wv_sb[:, kc, :],
                            start=False, stop=(kc == KC - 1))
                        if kc == KC - 1:
                            nc.vector.tensor_add(
                                va[:, mt, :, 0:D],
                                box[mt][:, 0:OUTC].rearrange(
                                    "p (h d) -> p h d", h=LH),
                                bv_rep[:])
                    return f

                for mt in range(4):
                    units.append(first(mt))
                    for kc in range(1, KC):
                        units.append(mid(mt, kc))
                return units

            def qkproj_units(nb, hh):
                """PE-filler closures for the q,k projection of (block, head)."""
                qk = qkpool.tile([D, 2, 512], BF16, tag=f"qk{hh}",
                                 name=f"qk{hh}")
                qk_tiles[nb][hh] = qk
                units = []
                box = {}

                def first(t):
                    m = 2 * hh + t

                    def f():
                        qps = ps_proj.tile([128, 512], F32, tag="proj",
                                           name="qps")
                        box[t] = qps
                        nc.tensor.matmul(
                            qps[0:D, :], wqk_sb[:, 0, m * D:(m + 1) * D],
                            xt_sb[nb][:, 0, :], start=True, stop=False)
                    return f

                def mid(t, kc):
                    m = 2 * hh + t

                    def f():
                        nc.tensor.matmul(
                            box[t][0:D, :], wqk_sb[:, kc, m * D:(m + 1) * D],
                            xt_sb[nb][:, kc, :],
                            start=False, stop=(kc == KC - 1))
                        if kc == KC - 1:
                            nc.vector.tensor_scalar_add(
                                qk[:, t, :], box[t][0:D, :],
                                bqk_sb[:, m:m + 1])
                    return f

                for t in range(2):
                    units.append(first(t))
                    for kc in range(1, KC):
                        units.append(mid(t, kc))
                return units

            def attn_head(Q, h):
                jmax = 4 * Q + 3
                P = 2 * Q + 2
                ops = ps_o.tile([D + 1, 512], F32, tag="o", name="ops")
                pt_l, info_l = {}, {}

                def S_pair(pr):
                    sps = ps_s.tile([128, 1024], F32, tag="s", name="sps")
                    info = []
                    col = 0  # pack the two j blocks adjacently: no garbage
                    for idx in range(2):
                        j = 2 * pr + idx
                        qoff = max(512 * Q, 128 * j)
                        width = 512 * (Q + 1) - qoff
                        info.append((j, qoff, width, col))
                        nc.tensor.matmul(
                            sps[:, col:col + width],
                            qk_tiles[j // 4][h][
                                :, 1, (j % 4) * 128:(j % 4) * 128 + 128],
                            qk_tiles[Q][h][
                                :, 0, qoff - 512 * Q:qoff - 512 * Q + width],
                            start=True, stop=True)
                        col += width
                    info_l[pr] = info
                    pt = ppool.tile([128, 1024], BF16, tag="p", name="pt")
                    pt_l[pr] = pt
                    nc.scalar.activation(
                        pt[:, 0:col], sps[:, 0:col],
                        mybir.ActivationFunctionType.Exp, scale=SCALE)
                    for j, qoff, width, c in info:
                        if j >= 4 * Q:  # diagonal block: causal mask
                            nc.gpsimd.affine_select(
                                out=pt[:, c:c + 128],
                                in_=pt[:, c:c + 128],
                                compare_op=mybir.AluOpType.is_ge,
                                fill=0.0, base=0, pattern=[[1, 128]],
                                channel_multiplier=-1)

                def PV_pair(pr):
                    for j, qoff, width, c in info_l[pr]:
                        nc.tensor.matmul(
                            ops[:, qoff - 512 * Q:512],
                            va_tiles[j // 4][:, j % 4, h, :],
                            pt_l[pr][:, c:c + width],
                            start=(j == 0), stop=(j == jmax))

                LAG = 3
                for pr in range(P):
                    S_pair(pr)
                    if pr == 1 and pend_o[0] is not None:
                        pend_o[0]()
                        pend_o[0] = None
                    if pr >= LAG:
                        PV_pair(pr - LAG)
                    weave()
                n_tail = 2 if P <= 3 else 1
                for k in range(max(0, P - LAG), P):
                    weave(n_tail)
                    PV_pair(k)

                def process_o():
                    o_sb = opool.tile([D + 1, 512], BF16, tag="osb",
                                      name="o_sb")
                    nc.vector.tensor_copy(o_sb[:], ops[:])
                    # row stride D+2 keeps each bf16 PSUM slice 4B-aligned
                    tps = ps_proj.tile([128, 4, D + 2], BF16, tag="proj",
                                       name="tps")
                    for t in range(4):
                        nc.tensor.transpose(
                            tps[:, t, 0:D + 1], o_sb[:, t * 128:(t + 1) * 128],
                            identity_bf[0:D + 1, 0:D + 1])
                    rr = rpool.tile([128, 4, 1], F32, tag="r", name="rr")
                    nc.vector.reciprocal(rr[:], tps[:, :, D:D + 1])
                    stage = spool.tile([128, 4, D], F32, tag="stage",
                                       name="stage")
                    for t in range(4):
                        nc.vector.tensor_scalar_mul(
                            stage[:, t, :], tps[:, t, 0:D], rr[:, t, :])
                    nc.gpsimd.dma_start(
                        out_v[Q, :, :, h * D:(h + 1) * D], stage[:])

                pend_o[0] = process_o

            # ---- body schedule ----
            load_x(0)
            load_x(1)
            for u in vproj_units(0):
                u()
            for hh in range(LH):
                for u in qkproj_units(0, hh):
                    u()
            for Q in range(NB):
                if Q + 2 < NB:
                    load_x(Q + 2)
                for h in range(LH):
                    if Q + 1 < NB:
                        if h == 0:
                            # qk of head 0 first: attention(Q+1) starts by
                            # reading it, so its eviction must not land at
                            # the very end of the drain
                            filler.extend(qkproj_units(Q + 1, 0))
                            filler.extend(vproj_units(Q + 1))
                        elif Q + 1 < NB - 1:
                            filler.extend(qkproj_units(Q + 1, h))
                        # when Q+1 is the last block, its qk head 1..3
                        # projections are deferred into that block itself
                    else:
                        if h + 1 < LH:
                            filler.extend(qkproj_units(Q, h + 1))
                    attn_head(Q, h)
                    if Q == NB - 1:
                        drain()
                drain()
            if pend_o[0] is not None:
                pend_o[0]()
                pend_o[0] = None

        if reps == 1:
            body()
        elif unroll:
            for _ in range(reps):
                body()
        else:
            with tc.For_i(0, reps, 1):
                body()
                if UNROLL2:
                    body()

    nc.compile()
    return nc


def f32r_round(a):
    """Round fp32 array to f32r precision (11-bit mantissa, RNE)."""
    u = np.ascontiguousarray(a, dtype=np.float32).view(np.uint32)
    u = (u + 0x7FF + ((u >> 12) & 1)) & np.uint32(0xFFFFF000)
    return u.view(np.float32)


def shard_inputs(x, W, b, use_f32r=True):
    """Full inputs -> per-core in_maps (numpy, fp32)."""
    x = np.asarray(x, dtype=np.float32)
    W = np.asarray(W, dtype=np.float32)
    b = np.asarray(b, dtype=np.float32)
    if use_f32r:
        # round once globally (elementwise, commutes with slicing below)
        x = f32r_round(x)
        W = f32r_round(W)
    rnd = lambda a: np.ascontiguousarray(a, dtype=np.float32)
    in_maps = []
    for c in range(NCORES):
        bc, g = divmod(c, 2)
        h0 = g * LH
        qcols = [W[:, 0 * C + (h0 + h) * D:0 * C + (h0 + h + 1) * D]
                 for h in range(LH)]
        kcols = [W[:, 1 * C + (h0 + h) * D:1 * C + (h0 + h + 1) * D]
                 for h in range(LH)]
        vcols = [W[:, 2 * C + (h0 + h) * D:2 * C + (h0 + h + 1) * D]
                 for h in range(LH)]
        wqk = np.concatenate(
            [m for h in range(LH) for m in (qcols[h], kcols[h])], axis=1)
        wv = np.concatenate(vcols, axis=1)
        bqk = np.stack(
            [b[t * C + (h0 + h) * D:t * C + (h0 + h + 1) * D]
             for h in range(LH) for t in (0, 1)], axis=1)
        bv = np.concatenate(
            [b[2 * C + (h0 + h) * D:2 * C + (h0 + h + 1) * D]
             for h in range(LH)])[None, :]
        in_maps.append({
            "xt": rnd(x[bc].T),
            "wqk": rnd(wqk),
            "wv": rnd(wv),
            "bqk": np.ascontiguousarray(bqk),
            "bv": rnd(bv),
        })
    return in_maps


def gather_outputs(results):
    """Per-core results -> full [B, N, C] output."""
    out = np.empty((B, N, C), dtype=np.float32)
    for c in range(NCORES):
        bc, g = divmod(c, 2)
        out[bc, :, g * OUTC:(g + 1) * OUTC] = results[c]["out"]
    return out


def kernel(x, W, b):
    nc = build(reps=1, use_f32r=True)
    in_maps = shard_inputs(x, W, b, use_f32r=True)
    res = run_bass_kernel_spmd(nc, in_maps, core_ids=list(range(NCORES)))
    return gather_outputs(res.results)


# revision 16
# speedup vs baseline: 1.7003x; 1.0325x over previous
"""Causal self-attention kernel for 8 Trainium2 NeuronCores.

Problem (hardcoded): x [4, 2048, 768] f32, W [768, 2304] f32, b [2304] f32.
reference: qkv = x@W+b; 8 heads, head_dim 96; causal softmax attention.

Sharding: core c handles batch c//2 and heads 4*(c%2) .. 4*(c%2)+3
(data-parallel over batch x tensor-parallel over heads). Host shards
inputs / gathers outputs around one SPMD NEFF; no device collectives.

Per-core device program (PE-pipelined schedule):
  - projection: qT,kT computed transposed [96, seq] in f32r (bias added on
    eviction), v computed natural [seq, 96] with a ones column appended so
    the PV matmul also produces the softmax denominator. Attention operands
    (q,k,v,P,o) are stored bf16 (full-rate PE for any width, cheap
    transposes); accumulation stays f32 in PSUM.
  - attention in S^T layout per 512-wide q block: S^T[k,q] = kT.T @ qT,
    exp without max subtraction (logits ~N(0,1)), causal = skip upper
    blocks + affine_select on diagonal blocks; o^T accumulated in PSUM,
    PE-transposed back and scaled by 1/denominator.
  - scheduling: the Tensor engine's clock ramps to 2.4 GHz only under
    sustained execution, so the emission order keeps PE gapless: S and PV
    matmuls are software-pipelined with a 2-pair lag (PV(k) is emitted
    after S(k+2), hiding the exp->mask latency), the next block's
    projection matmuls are woven one-at-a-time into the attention pair
    loop as PE filler (covering the ACT engine's exp deficit), and each
    head's o-postprocessing is deferred into the next head's pair loop.
    Input DMAs issue from the sync queue, output DMAs from the gpsimd
    queue, so input prefetch is never stuck behind output drains.

Matmul inputs for the projection use float32r (fp32 storage, 11-bit
mantissa); host pre-rounds those inputs to f32r precision.
"""

import functools
from collections import deque
from contextlib import ExitStack

import numpy as np

import concourse.bacc as bacc
import concourse.bass as bass
import concourse.mybir as mybir
import concourse.tile as tile
from concourse.bass_utils import run_bass_kernel_spmd
from concourse.masks import make_identity

F32 = mybir.dt.float32
F32R = mybir.dt.float32r
BF16 = mybir.dt.bfloat16

B, N, C, H = 4, 2048, 768, 8
D = C // H            # 96
NCORES = 8
LH = 4                # local heads per core
KC = C // 128         # 6 contraction chunks
NB = N // 512         # 4 seq blocks of 512
OUTC = LH * D         # 384
SCALE = float(1.0 / np.sqrt(np.float32(D)))
UNROLL2 = False
LAG = 3               # S->PV software-pipeline depth (pairs)


@functools.lru_cache(maxsize=8)
def build(reps=1, use_f32r=True, unroll=False):
    MDT = F32R if use_f32r else F32
    nc = bacc.Bacc("TRN2", target_bir_lowering=False, debug=False,
                   num_devices=NCORES)
    xt_d = nc.dram_tensor("xt", [C, N], MDT, kind="ExternalInput")
    wqk_d = nc.dram_tensor("wqk", [C, 2 * LH * D], MDT, kind="ExternalInput")
    wv_d = nc.dram_tensor("wv", [C, LH * D], MDT, kind="ExternalInput")
    bqk_d = nc.dram_tensor("bqk", [D, 2 * LH], F32, kind="ExternalInput")
    bv_d = nc.dram_tensor("bv", [1, LH * D], MDT, kind="ExternalInput")
    out_d = nc.dram_tensor("out", [N, OUTC], F32, kind="ExternalOutput")

    xt_v = xt_d.ap().rearrange("(kc p) n -> p kc n", p=128)
    wqk_v = wqk_d.ap().rearrange("(kc p) m -> p kc m", p=128)
    wv_v = wv_d.ap().rearrange("(kc p) m -> p kc m", p=128)
    out_v = out_d.ap().rearrange("(qq t p) c -> qq p t c", t=4, p=128)

    with tile.TileContext(nc) as tc, ExitStack() as ctx:
        const = ctx.enter_context(tc.tile_pool(name="const", bufs=1))
        wpool = ctx.enter_context(tc.tile_pool(name="w", bufs=1))
        xpool = ctx.enter_context(tc.tile_pool(name="x", bufs=3))
        qkpool = ctx.enter_context(tc.tile_pool(name="qk", bufs=NB + 1))
        vpool = ctx.enter_context(tc.tile_pool(name="vaug", bufs=NB + 1))
        ppool = ctx.enter_context(tc.tile_pool(name="p", bufs=6))
        opool = ctx.enter_context(tc.tile_pool(name="osb", bufs=3))
        rpool = ctx.enter_context(tc.tile_pool(name="r", bufs=4))
        spool = ctx.enter_context(tc.tile_pool(name="stage", bufs=3))
        ps_proj = ctx.enter_context(
            tc.tile_pool(name="ps_proj", bufs=2, space="PSUM"))
        ps_s = ctx.enter_context(
            tc.tile_pool(name="ps_s", bufs=2, space="PSUM"))
        ps_o = ctx.enter_context(
            tc.tile_pool(name="ps_o", bufs=2, space="PSUM"))

        # one-time constants
        identity_bf = const.tile([128, 128], BF16)
        make_identity(nc, identity_bf[:])
        ones = const.tile([1, 128], F32)
        nc.gpsimd.memset(ones[:], 1.0)
        ones_r = const.tile([1, 128], MDT)
        nc.vector.tensor_copy(ones_r[:], ones[:])

        wqk_sb = wpool.tile([128, KC, 2 * LH * D], MDT, tag="wqk")
        wv_sb = wpool.tile([128, KC, LH * D], MDT, tag="wv")
        for kc in range(KC):
            nc.sync.dma_start(wqk_sb[:, kc, :], wqk_v[:, kc, :])
            nc.sync.dma_start(wv_sb[:, kc, :], wv_v[:, kc, :])
        bqk_sb = wpool.tile([D, 2 * LH], F32, tag="bqk")
        nc.sync.dma_start(bqk_sb[:], bqk_d.ap())
        bv_sb = wpool.tile([1, LH * D], MDT, tag="bv")
        nc.sync.dma_start(bv_sb[:], bv_d.ap())
        # bv broadcast to all 128 partitions (once) so the v bias can be
        # applied during PSUM eviction instead of a per-block matmul
        bv_ps = ps_proj.tile([128, 512], F32, tag="proj", name="bv_ps")
        nc.tensor.matmul(bv_ps[:, 0:OUTC], ones_r[:, :], bv_sb[:, :],
                         start=True, stop=True)
        bv_rep = wpool.tile([128, LH, D], F32, tag="bvrep")
        nc.vector.tensor_copy(
            bv_rep[:], bv_ps[:, 0:OUTC].rearrange("p (h d) -> p h d", h=LH))

        def body():
            filler = deque()

            def weave(n=1):
                for _ in range(n):
                    if filler:
                        filler.popleft()()

            def drain():
                while filler:
                    filler.popleft()()

            xt_sb = [None] * NB
            va_tiles = [None] * NB
            qk_tiles = [[None] * LH for _ in range(NB)]
            pend_o = [None]

            def load_x(nb):
                t = xpool.tile([128, KC, 512], MDT, tag="xt", name="xt_sb")
                for kc in range(KC):
                    nc.sync.dma_start(
                        t[:, kc, :], xt_v[:, kc, nb * 512:(nb + 1) * 512])
                xt_sb[nb] = t

            def vproj_units(nb):
                """PE-filler closures for the v projection of seq block nb."""
                va = vpool.tile([128, 4, LH, D + 1], BF16, tag="va", name="va")
                va_tiles[nb] = va
                units = []
                box = {}

                def first(mt):
                    def f():
                        if mt == 0:
                            nc.gpsimd.memset(va[:, :, :, D:D + 1], 1.0)
                        vps = ps_proj.tile([128, 512], F32, tag="proj",
                                           name="vps")
                        box[mt] = vps
                        nc.tensor.matmul(
                            vps[:, 0:OUTC],
                            xt_sb[nb][:, 0, mt * 128:(mt + 1) * 128],
                            wv_sb[:, 0, :], start=True, stop=False)
                    return f

                def mid(mt, kc):
                    def f():
                        nc.tensor.matmul(
                            box[mt][:, 0:OUTC],
                            xt_sb[nb][:, kc, mt * 128:(mt + 1) * 128],
                            wv_sb[:, kc, :],
                            start=False, stop=(kc == KC - 1))
                        if kc == KC - 1:
                            nc.vector.tensor_add(
                                va[:, mt, :, 0:D],
                                box[mt][:, 0:OUTC].rearrange(
                                    "p (h d) -> p h d", h=LH),
                                bv_rep[:])
                    return f

                for mt in range(4):
                    units.append(first(mt))
                    for kc in range(1, KC):
                        units.append(mid(mt, kc))
                return units

            def qkproj_units(nb, hh):
                """PE-filler closures for the q,k projection of (block, head)."""
                qk = qkpool.tile([D, 2, 512], BF16, tag=f"qk{hh}",
                                 name=f"qk{hh}")
                qk_tiles[nb][hh] = qk
                units = []
                box = {}

                def first(t):
                    m = 2 * hh + t

                    def f():
                        qps = ps_proj.tile([128, 512], F32, tag="proj",
                                           name="qps")
                        box[t] = qps
                        nc.tensor.matmul(
                            qps[0:D, :], wqk_sb[:, 0, m * D:(m + 1) * D],
                            xt_sb[nb][:, 0, :], start=True, stop=False)
                    return f

                def mid(t, kc):
                    m = 2 * hh + t

                    def f():
                        nc.tensor.matmul(
                            box[t][0:D, :], wqk_sb[:, kc, m * D:(m + 1) * D],
                            xt_sb[nb][:, kc, :],
                            start=False, stop=(kc == KC - 1))
                        if kc == KC - 1:
                            nc.vector.tensor_scalar_add(
                                qk[:, t, :], box[t][0:D, :],
                                bqk_sb[:, m:m + 1])
                    return f

                for t in range(2):
                    units.append(first(t))
                    for kc in range(1, KC):
                        units.append(mid(t, kc))
                return units

            def attn_head(Q, h):
                jmax = 4 * Q + 3
                P = 2 * Q + 2
                ops = ps_o.tile([D + 1, 512], F32, tag="o", name="ops")
                pt_l, info_l = {}, {}

                def S_pair(pr):
                    sps = ps_s.tile([128, 1024], F32, tag="s", name="sps")
                    info = []
                    col = 0  # pack the two j blocks adjacently: no garbage
                    for idx in range(2):
                        j = 2 * pr + idx
                        qoff = max(512 * Q, 128 * j)
                        width = 512 * (Q + 1) - qoff
                        info.append((j, qoff, width, col))
                        nc.tensor.matmul(
                            sps[:, col:col + width],
                            qk_tiles[j // 4][h][
                                :, 1, (j % 4) * 128:(j % 4) * 128 + 128],
                            qk_tiles[Q][h][
                                :, 0, qoff - 512 * Q:qoff - 512 * Q + width],
                            start=True, stop=True)
                        col += width
                    info_l[pr] = info
                    pt = ppool.tile([128, 1024], BF16, tag="p", name="pt")
                    pt_l[pr] = pt
                    nc.scalar.activation(
                        pt[:, 0:col], sps[:, 0:col],
                        mybir.ActivationFunctionType.Exp, scale=SCALE)
                    for j, qoff, width, c in info:
                        if j >= 4 * Q:  # diagonal block: causal mask
                            nc.gpsimd.affine_select(
                                out=pt[:, c:c + 128],
                                in_=pt[:, c:c + 128],
                                compare_op=mybir.AluOpType.is_ge,
                                fill=0.0, base=0, pattern=[[1, 128]],
                                channel_multiplier=-1)

                def PV_pair(pr):
                    for j, qoff, width, c in info_l[pr]:
                        nc.tensor.matmul(
                            ops[:, qoff - 512 * Q:512],
                            va_tiles[j // 4][:, j % 4, h, :],
                            pt_l[pr][:, c:c + width],
                            start=(j == 0), stop=(j == jmax))

                for pr in range(P):
                    S_pair(pr)
                    if pr == 1 and pend_o[0] is not None:
                        pend_o[0]()
                        pend_o[0] = None
                    if pr >= LAG:
                        PV_pair(pr - LAG)
                    weave()
                n_tail = 2 if P <= 3 else 1
                for k in range(max(0, P - LAG), P):
                    weave(n_tail)
                    PV_pair(k)

                def process_o():
                    o_sb = opool.tile([D + 1, 512], BF16, tag="osb",
                                      name="o_sb")
                    nc.vector.tensor_copy(o_sb[:], ops[:])
                    # row stride D+2 keeps each bf16 PSUM slice 4B-aligned
                    tps = ps_proj.tile([128, 4, D + 2], BF16, tag="proj",
                                       name="tps")
                    for t in range(4):
                        nc.tensor.transpose(
                            tps[:, t, 0:D + 1], o_sb[:, t * 128:(t + 1) * 128],
                            identity_bf[0:D + 1, 0:D + 1])
                    rr = rpool.tile([128, 4, 1], F32, tag="r", name="rr")
                    nc.vector.reciprocal(rr[:], tps[:, :, D:D + 1])
                    stage = spool.tile([128, 4, D], F32, tag="stage",
                                       name="stage")
                    for t in range(4):
                        nc.vector.tensor_scalar_mul(
                            stage[:, t, :], tps[:, t, 0:D], rr[:, t, :])
                    # triggered from the SP queue, not gpsimd: a trigger
                    # dep-waits in-queue for its stage tile, which on the
                    # Pool queue would block the causal masks gating PV
                    nc.sync.dma_start(
                        out_v[Q, :, :, h * D:(h + 1) * D], stage[:])

                pend_o[0] = process_o

            # ---- body schedule ----
            load_x(0)
            load_x(1)
            for u in vproj_units(0):
                u()
            for hh in range(LH):
                for u in qkproj_units(0, hh):
                    u()
            for Q in range(NB):
                if Q + 2 < NB:
                    load_x(Q + 2)
                for h in range(LH):
                    if Q + 1 < NB:
                        if h == 0:
                            # qk of head 0 first: attention(Q+1) starts by
                            # reading it, so its eviction must not land at
                            # the very end of the drain
                            filler.extend(qkproj_units(Q + 1, 0))
                            filler.extend(vproj_units(Q + 1))
                        elif Q + 1 < NB - 1:
                            filler.extend(qkproj_units(Q + 1, h))
                        # when Q+1 is the last block, its qk head 1..3
                        # projections are deferred into that block itself
                    else:
                        if h + 1 < LH:
                            filler.extend(qkproj_units(Q, h + 1))
                    attn_head(Q, h)
                    if Q == NB - 1:
                        drain()
                drain()
            if pend_o[0] is not None:
                pend_o[0]()
                pend_o[0] = None

        if reps == 1:
            body()
        elif unroll:
            for _ in range(reps):
                body()
        else:
            with tc.For_i(0, reps, 1):
                body()
                if UNROLL2:
                    body()

    nc.compile()
    return nc


def f32r_round(a):
    """Round fp32 array to f32r precision (11-bit mantissa, RNE)."""
    u = np.ascontiguousarray(a, dtype=np.float32).view(np.uint32)
    u = (u + 0x7FF + ((u >> 12) & 1)) & np.uint32(0xFFFFF000)
    return u.view(np.float32)


def shard_inputs(x, W, b, use_f32r=True):
    """Full inputs -> per-core in_maps (numpy, fp32)."""
    x = np.asarray(x, dtype=np.float32)
    W = np.asarray(W, dtype=np.float32)
    b = np.asarray(b, dtype=np.float32)
    if use_f32r:
        # round once globally (elementwise, commutes with slicing below)
        x = f32r_round(x)
        W = f32r_round(W)
    rnd = lambda a: np.ascontiguousarray(a, dtype=np.float32)
    in_maps = []
    for c in range(NCORES):
        bc, g = divmod(c, 2)
        h0 = g * LH
        qcols = [W[:, 0 * C + (h0 + h) * D:0 * C + (h0 + h + 1) * D]
                 for h in range(LH)]
        kcols = [W[:, 1 * C + (h0 + h) * D:1 * C + (h0 + h + 1) * D]
                 for h in range(LH)]
        vcols = [W[:, 2 * C + (h0 + h) * D:2 * C + (h0 + h + 1) * D]
                 for h in range(LH)]
        wqk = np.concatenate(
            [m for h in range(LH) for m in (qcols[h], kcols[h])], axis=1)
        wv = np.concatenate(vcols, axis=1)
        bqk = np.stack(
            [b[t * C + (h0 + h) * D:t * C + (h0 + h + 1) * D]
             for h in range(LH) for t in (0, 1)], axis=1)
        bv = np.concatenate(
            [b[2 * C + (h0 + h) * D:2 * C + (h0 + h + 1) * D]
             for h in range(LH)])[None, :]
        in_maps.append({
            "xt": rnd(x[bc].T),
            "wqk": rnd(wqk),
            "wv": rnd(wv),
            "bqk": np.ascontiguousarray(bqk),
            "bv": rnd(bv),
        })
    return in_maps


def gather_outputs(results):
    """Per-core results -> full [B, N, C] output."""
    out = np.empty((B, N, C), dtype=np.float32)
    for c in range(NCORES):
        bc, g = divmod(c, 2)
        out[bc, :, g * OUTC:(g + 1) * OUTC] = results[c]["out"]
    return out


def kernel(x, W, b):
    nc = build(reps=1, use_f32r=True)
    in_maps = shard_inputs(x, W, b, use_f32r=True)
    res = run_bass_kernel_spmd(nc, in_maps, core_ids=list(range(NCORES)))
    return gather_outputs(res.results)
